# revision 1
# baseline (speedup 1.0000x reference)
"""Trainium2 Bass kernel for nn_BaseMultiHeadAttention (B=2, S=2048, E=1024, H=16).

Sharding: tensor-parallel over heads — each of the 8 NeuronCores handles 2
heads for both batch elements (4 (b,h) jobs/core).  RMSNorm + RoPE + causal
attention run per-head on-device; the output projection is row-sharded
(each core contracts its 128 ctx features against proj_w), and the host
sums the 8 partial [B,S,E] outputs (the all-reduce) and adds the bias.

Device pipeline per core:
  Phase A (per job): load q/k [128, NT*64] (host pre-arranged
    partition-major so every DMA descriptor is a 4KB contiguous run),
    sum-of-squares -> sqrt(mean+eps) on ACT -> reciprocal on DVE, then
    batched RoPE (host permutes q/k features to de-interleave the rope
    pairs — a consistent permutation leaves q.k dot products unchanged,
    so every rope op is a contiguous full-width DVE op), then
    PE-transpose to qT/kT [64, S] (float32r) for the attention matmuls.
  Phase B (per b, q-chunk of 512, head): scoresT[k,q] = kT.T @ qT on PE in
    float32r (1 cyc/row), causal block-sparse (only k-blocks <= chunk end),
    triangular mask add on diagonal blocks (DVE), exp via ACT over
    [128,1024] PSUM groups (scale=D^-0.5 folded in; no max subtraction:
    RMSNorm bounds |scores*scale| <= ~8), ctx = p.T @ [v|1] accumulated in
    PSUM (the ones column yields softmax row-sums for free), rows scaled by
    1/sum on the PSUM->SBUF copy.  After both heads: PE-transpose ctx
    [s,128f] -> [128f,s], partial projection in float32r, DMA out.
  A and B share PSUM pools and are interleaved A0 A1 B0 A2 A3 B1 so the
  scheduler overlaps phase A of later jobs under phase B compute.
"""
import numpy as np

import bass_rust
import concourse.bass as bass
import concourse.mybir as mybir
import concourse.tile as tile
from concourse.bass_utils import run_bass_kernel_spmd
from concourse.masks import make_identity

B, S, E, H, D = 2, 2048, 1024, 16, 64
HD = D // 2
N_CORES = 8
HL = H // N_CORES          # 2 heads per core
NJ = B * HL                # 4 (b, h) jobs per core
NT = S // 128              # 16 s-tiles per job
NCH = S // 512             # 4 q-chunks per job
EPS = 1.1920928955078125e-07
SCALE = float(D) ** -0.5
NEG = -1e30
f32 = mybir.dt.float32
f32r = mybir.dt.float32r
ALU = mybir.AluOpType
ACTF = mybir.ActivationFunctionType

# float32r runs the PE at 1 cycle/row for moving dims >= 256 (vs fp32's 4).
F32R_SCORES = True
F32R_PROJ = True
# fp16 for the attention-weights @ v matmul: PE 1 cyc/row (vs fp32's 4) with
# a 10-bit mantissa; unnormalized p = exp(s) <= e^8 fits fp16 range.
CTX_BF16 = True
bf16 = mybir.dt.float16

_TC = tile.TileContext


def _legalize_waits(nc):
    """Split multi-wait sync_infos for this walrus build.

    This neuronxcc's codegen allows 1 sync wait per instruction (2 on
    EventSemaphore), while the Tile scheduler attaches all outstanding
    waits to one instruction.  Hoist the excess onto same-engine NoOps
    inserted immediately before the offending instruction — the engine
    executes its stream in order, so blocking semantics are identical.
    """
    uid = 0
    for f in nc.m.functions:
        for blk in f.blocks:
            insts = list(blk.instructions)
            out, changed = [], False
            for inst in insts:
                si = inst.sync_info
                cap = 2 if isinstance(inst, mybir.InstEventSemaphore) else 1
                if si is not None and len(si.on_wait) > cap:
                    changed = True
                    waits = list(si.on_wait)
                    for w in waits[:-cap]:
                        carrier = mybir.InstNoOp(
                            name=f"legwait-{uid}", engine=inst.engine,
                            ins=[], outs=[])
                        uid += 1
                        carrier.sync_info = bass_rust.SyncInfo(
                            on_wait=[w], on_update=[])
                        nc.register_instruction(carrier, overwrite=True)
                        out.append(carrier)
                    si.on_wait = waits[-cap:]
                    inst.sync_info = si
                out.append(inst)
            if changed:
                blk.instructions = out


def build_nc():
    nc = bass.Bass("TRN2", target_bir_lowering=False, debug=False)
    q_in = nc.dram_tensor("q", [NJ, 128, NT, D], f32, kind="ExternalInput")
    k_in = nc.dram_tensor("k", [NJ, 128, NT, D], f32, kind="ExternalInput")
    v_in = nc.dram_tensor("v", [NJ, 128, NT, D], f32, kind="ExternalInput")
    cos_in = nc.dram_tensor("cos", [128, NT, HD], f32, kind="ExternalInput")
    sin_in = nc.dram_tensor("sin", [128, NT, HD], f32, kind="ExternalInput")
    wt_in = nc.dram_tensor("wt", [128, E], f32, kind="ExternalInput")
    out = nc.dram_tensor("out", [B * S, E], f32, kind="ExternalOutput")

    with _TC(nc) as tc:
        with tc.tile_pool(name="const", bufs=1) as cp, \
             tc.tile_pool(name="pa", bufs=2) as pa, \
             tc.tile_pool(name="pb", bufs=2) as pb, \
             tc.tile_pool(name="pp", bufs=16) as pp, \
             tc.tile_pool(name="po", bufs=4) as po, \
             tc.tile_pool(name="ps_s", bufs=2, space="PSUM") as ps_s, \
             tc.tile_pool(name="ps_sm", bufs=2, space="PSUM") as ps_sm, \
             tc.tile_pool(name="ps_o", bufs=2, space="PSUM") as ps_o:
            ident = cp.tile([128, 128], f32)
            make_identity(nc, ident)
            eps_t = cp.tile([128, 1], f32)
            nc.vector.memset(eps_t, EPS)
            cos_sb = cp.tile([128, NT, HD], f32)
            sin_sb = cp.tile([128, NT, HD], f32)
            wt_sb = cp.tile([128, E], f32r if F32R_PROJ else f32)
            wt_raw = cp.tile([128, E], f32)
            qT = cp.tile([64, NJ, S], f32r if F32R_SCORES else f32)
            kT = cp.tile([64, NJ, S], f32r if F32R_SCORES else f32)
            vsb = cp.tile([128, NJ, NT, D + 1],
                          bf16 if CTX_BF16 else f32)

            # ------------- Phase A: norm + rope + transpose (per job) -------
            def phase_a(j, nsub=1):
                # gpsimd is idle in the phase-A region; use it for jobs whose
                # latency is hidden under phase-B compute (not job 0)
                off = nc.gpsimd if j > 0 else nc.any
                # nsub > 1 shortens the pipeline-fill latency (used for the
                # first job, which nothing else overlaps with)
                NS = NT // nsub
                for dstT, src in ((qT, q_in), (kT, k_in)):
                    for sub in range(nsub):
                        tsl = slice(sub * NS, (sub + 1) * NS)
                        raw = pa.tile([128, NS, D], f32, tag="raw", name="raw")
                        nc.sync.dma_start(out=raw, in_=src.ap()[j][:, tsl])
                        sq = pa.tile([128, NS, D], f32, tag="sq", name="sq")
                        off.tensor_mul(sq, raw, raw)
                        ss = pa.tile([128, NS], f32, tag="ss", name="ss")
                        nc.vector.reduce_sum(ss, sq, axis=mybir.AxisListType.X)
                        rs = pa.tile([128, NS], f32, tag="rs", name="rs")
                        nc.scalar.activation(
                            out=rs, in_=ss, func=ACTF.Sqrt,
                            bias=eps_t, scale=1.0 / D,
                        )
                        nc.vector.reciprocal(out=rs, in_=rs)
                        # normalize: raw * rs (broadcast over d via step-0 AP)
                        rs_b = bass.AP(
                            tensor=rs.tensor, offset=rs.offset,
                            ap=[list(rs.ap[0]), list(rs.ap[1]), [0, D]])
                        xn = pa.tile([128, NS, D], f32, tag="xn", name="xn")
                        nc.any.tensor_mul(xn, raw, rs_b)
                        x1, x2 = xn[:, :, 0:HD], xn[:, :, HD:D]
                        rn_all = pa.tile([128, NS, D], f32, tag="rn",
                                         name="rn")
                        t1 = pa.tile([128, NS, HD], f32, tag="t1", name="t1")
                        t2 = pa.tile([128, NS, HD], f32, tag="t2", name="t2")
                        csl, ssl = cos_sb[:, tsl, :], sin_sb[:, tsl, :]
                        off.tensor_mul(t1, x1, csl)
                        nc.any.tensor_mul(t2, x2, ssl)
                        nc.any.tensor_sub(rn_all[:, :, 0:HD], t1, t2)
                        off.tensor_mul(t1, x1, ssl)
                        nc.any.tensor_mul(t2, x2, csl)
                        nc.any.tensor_add(rn_all[:, :, HD:D], t1, t2)
                        for tg in range(NS // 4):
                            ps_tr = ps_o.tile([64, 512], f32, tag="o",
                                              name="ps_tr")
                            for tt in range(4):
                                t = tg * 4 + tt
                                nc.tensor.transpose(
                                    ps_tr[:, tt * 128:(tt + 1) * 128],
                                    rn_all[:, t, :], ident)
                            base = (sub * NS + tg * 4) * 128
                            nc.any.tensor_copy(
                                dstT[:, j, base:base + 512], ps_tr)

            # v is only needed by the ctx matmuls, well after q/k
            def load_v(j):
                nc.gpsimd.memset(vsb[:, j, :, :], 1.0)
                if CTX_BF16:
                    vraw = pa.tile([128, NT, D], f32, tag="vraw", name="vraw")
                    nc.sync.dma_start(out=vraw, in_=v_in.ap()[j])
                    nc.gpsimd.tensor_copy(vsb[:, j, :, 0:D], vraw)
                else:
                    nc.sync.dma_start(out=vsb[:, j, :, 0:D], in_=v_in.ap()[j])

            # ------------- Phase B: attention + projection (per batch) ------
            def phase_b(b, chunks=None):
                for c in chunks if chunks is not None else range(NCH):
                    cpair = [pb.tile([128, HL * D], f32, tag=f"cpair{qb}",
                                     name=f"cpair{qb}")
                             for qb in range(4)]
                    for hl in range(HL):
                        j = b * HL + hl
                        ngrp = 2 * c + 2
                        ptiles = []
                        for g in range(ngrp):
                            sps = ps_s.tile([128, 1024], f32, tag="s",
                                            name="sps")
                            for u in range(2):
                                jj = 2 * g + u
                                nc.tensor.matmul(
                                    sps[:, u * 512:(u + 1) * 512],
                                    lhsT=kT[:, j, jj * 128:(jj + 1) * 128],
                                    rhs=qT[:, j, c * 512:(c + 1) * 512],
                                    start=True, stop=True,
                                )
                            pt = pp.tile([128, 1024],
                                         bf16 if CTX_BF16 else f32,
                                         tag="p", name="pt")
                            nc.scalar.activation(
                                out=pt, in_=sps, func=ACTF.Exp, scale=SCALE)
                            for u in range(2):
                                jj = 2 * g + u
                                if jj >= 4 * c:
                                    # diag block: zero the non-causal (q < k)
                                    # part of p multiplicatively
                                    tl = jj - 4 * c
                                    sl = pt[:, u * 512 + tl * 128:
                                            u * 512 + (tl + 1) * 128]
                                    nc.gpsimd.affine_select(
                                        out=sl, in_=sl, compare_op=ALU.is_ge,
                                        fill=0.0, base=0, pattern=[[1, 128]],
                                        channel_multiplier=-1)
                            ptiles.append(pt)
                        ctx = ps_sm.tile([128, 4, D + 1], f32, tag="sm",
                                         name="ctx")
                        for qb in range(4):
                            i = 4 * c + qb
                            for jj in range(i + 1):
                                nc.tensor.matmul(
                                    ctx[:, qb, :],
                                    lhsT=ptiles[jj // 2][
                                        :, (jj % 2) * 512 + qb * 128:
                                           (jj % 2) * 512 + (qb + 1) * 128],
                                    rhs=vsb[:, j, jj, :],
                                    start=(jj == 0), stop=(jj == i),
                                )
                            rsum = pb.tile([128, 1], f32, tag="rsum",
                                           name="rsum")
                            nc.vector.reciprocal(out=rsum,
                                                 in_=ctx[:, qb, D:D + 1])
                            nc.vector.tensor_scalar_mul(
                                cpair[qb][:, hl * D:(hl + 1) * D],
                                ctx[:, qb, 0:D], rsum)
                    for qb in range(4):
                        i = 4 * c + qb
                        trp = ps_sm.tile([128, 128], f32, tag="sm", name="trp")
                        nc.tensor.transpose(trp, cpair[qb], ident)
                        ctxT = pb.tile([128, 128], f32r if F32R_PROJ else f32,
                                       tag="ctxT", name="ctxT")
                        nc.any.tensor_copy(ctxT, trp)
                        osb = po.tile([128, E], f32, tag="osb", name="osb")
                        for n in range(2):
                            ops_ = ps_o.tile([128, 512], f32, tag="o",
                                             name="ops")
                            nc.tensor.matmul(
                                ops_, lhsT=ctxT,
                                rhs=wt_sb[:, n * 512:(n + 1) * 512],
                                start=True, stop=True,
                            )
                            nc.any.tensor_copy(
                                osb[:, n * 512:(n + 1) * 512], ops_)
                        nc.sync.dma_start(
                            out=out.ap()[b * S + i * 128:
                                         b * S + (i + 1) * 128, :],
                            in_=osb,
                        )

            nc.sync.dma_start(out=cos_sb, in_=cos_in.ap())
            nc.sync.dma_start(out=sin_sb, in_=sin_in.ap())
            phase_a(0)
            nc.sync.dma_start(out=wt_raw, in_=wt_in.ap())
            nc.any.tensor_copy(wt_sb, wt_raw)
            phase_a(1)
            load_v(0)
            load_v(1)
            phase_b(0, [3])
            phase_a(2)
            load_v(2)
            phase_b(0, [2])
            phase_a(3)
            load_v(3)
            phase_b(0, [1])
            phase_b(1, [3])
            phase_b(0, [0])
            phase_b(1, [2, 1, 0])
    _legalize_waits(nc)
    return nc


# even rope lanes first, then odd — see Phase A comment
_ROPE_PERM = np.concatenate([np.arange(0, D, 2), np.arange(1, D, 2)])


def _shard_inputs(q, k, v, cos, sin, proj_w):
    """Per-core input maps (host-side layout prep only — no module math)."""
    qh = q.reshape(B, S, H, D)
    kh = k.reshape(B, S, H, D)
    vh = v.reshape(B, S, H, D)
    # [S, HD] -> [128, NT, HD] partition-major
    cos_t = np.ascontiguousarray(
        cos.reshape(NT, 128, HD).transpose(1, 0, 2), np.float32)
    sin_t = np.ascontiguousarray(
        sin.reshape(NT, 128, HD).transpose(1, 0, 2), np.float32)
    maps = []
    for core in range(N_CORES):
        hs = slice(HL * core, HL * (core + 1))

        def tiles(x, permute):
            xs = x[:, :, hs, :].transpose(0, 2, 1, 3)  # [B, HL, S, D]
            if permute:
                xs = xs[..., _ROPE_PERM]
            # [NJ, NT, 128, D] -> [NJ, 128, NT, D] partition-major
            return np.ascontiguousarray(
                xs.reshape(NJ, NT, 128, D).transpose(0, 2, 1, 3), np.float32)

        wt_c = np.ascontiguousarray(
            proj_w[:, 128 * core:128 * (core + 1)].T, np.float32)
        maps.append({
            "q": tiles(qh, True), "k": tiles(kh, True),
            "v": tiles(vh, False),
            "cos": cos_t, "sin": sin_t, "wt": wt_c,
        })
    return maps


_NC_CACHE = []


def _get_nc():
    if not _NC_CACHE:
        _NC_CACHE.append(build_nc())
    return _NC_CACHE[0]


def kernel(q, k, v, attn_mask, padding_mask, qn_w, kn_w, proj_w, proj_b,
           cos, sin):
    q = np.asarray(q, np.float32)
    k = np.asarray(k, np.float32)
    v = np.asarray(v, np.float32)
    proj_w = np.asarray(proj_w, np.float32)
    proj_b = np.asarray(proj_b, np.float32)
    cos = np.asarray(cos, np.float32)
    sin = np.asarray(sin, np.float32)
    attn_mask = np.asarray(attn_mask)
    padding_mask = np.asarray(padding_mask)
    qn_w = np.asarray(qn_w, np.float32)
    kn_w = np.asarray(kn_w, np.float32)
    # The kernel bakes in: causal attn_mask, no padding, unit RMSNorm weights.
    assert np.array_equal(
        attn_mask.reshape(S, S), np.tril(np.ones((S, S), attn_mask.dtype)))
    assert padding_mask.all()
    assert np.all(qn_w == 1.0) and np.all(kn_w == 1.0)

    in_maps = _shard_inputs(q, k, v, cos, sin, proj_w)
    nc = _get_nc()
    res = run_bass_kernel_spmd(nc, in_maps, core_ids=list(range(N_CORES)))
    parts = np.stack([r["out"] for r in res.results])      # [8, B*S, E]
    full = parts.sum(axis=0, dtype=np.float32) + proj_b[None, :]
    return full.reshape(B, S, E).astype(np.float32)



# revision 8
# speedup vs baseline: 1.1213x; 1.1213x over previous
"""Trainium2 Bass kernel for nn_BaseMultiHeadAttention (B=2, S=2048, E=1024, H=16).

Sharding: tensor-parallel over heads - each of the 8 NeuronCores handles 2
heads for both batch elements.  RMSNorm + RoPE + causal attention run
per-head on-device; the output projection is row-sharded (each core
contracts its 128 ctx features against proj_w) and the host sums the 8
fp16 partial [B*S, E] outputs (the all-reduce) and adds the bias.

Key layout/perf choices (vs the f32 baseline):
  * All device inputs are fp16 (host casts; ~5e-4 rel err, well inside the
    2e-2 gate).  PE matmuls run at 1 cyc/row at any moving size, DVE
    elementwise ops with all-fp16 operands run at 2x, transposes write fp16
    PSUM so the PSUM->SBUF copies also run at 2x.
  * Both heads are packed in the feature dim: tiles are [128 s, NT, 128]
    where 128 = 2 heads x 64 rope-permuted features; a single PE transpose
    per s-tile yields qT/kT with head h on partitions [64h, 64h+64).
  * RMSNorm: sum-of-squares is rotation-invariant, so k is roped
    unnormalized and its 1/rms (x softmax 1/sqrt(D), folded via
    sqrt(ss + D*eps)) is applied as the per-partition scale AP of the Exp
    activation - k normalization costs zero elementwise work.  q gets one
    fused scale multiply after rope.
  * Scores/softmax at [1 k-tile x 1024 q-chunk] granularity: exact causal
    trimming of both the score matmuls and the exp widths; p = exp(scores)
    unnormalized in fp16 (bounded by e^8), with the ones-column of v giving
    softmax row-sums inside the ctx matmul; 1/rowsum is fused into the
    mandatory ctx PSUM->SBUF copy.
  * Output projection partials are written as fp16 (halves the 16MB
    output DMA); the PSUM->SBUF output copies are split between DVE and
    Pool to balance engine load under the ACT(exp) roofline.
"""
import numpy as np

import bass_rust
import concourse.bass as bass
import concourse.mybir as mybir
import concourse.tile as tile
from concourse.bass_utils import run_bass_kernel_spmd
from concourse.masks import make_identity

B, S, E, H, D = 2, 2048, 1024, 16, 64
HD = D // 2                # 32 rope pair count
N_CORES = 8
HL = H // N_CORES          # 2 heads per core
D2 = HL * D                # 128 packed feature dim
NT = S // 128              # 16 s-tiles
NCH = 2                    # q-chunks of 1024
CW = S // NCH              # 1024 chunk width
KT = S // 128              # 16 k-tiles
EPS = 1.1920928955078125e-07
f32 = mybir.dt.float32
f16 = mybir.dt.float16
ALU = mybir.AluOpType
ACTF = mybir.ActivationFunctionType

# engine-balance knobs (DVE vs Pool)
REDUCE_ON_POOL = False     # phase-A sum-of-squares reduce (DVE only op)
DIAG_ON_POOL = True        # diagonal-tile causal zeroing of p
OSB_POOL_PATTERN = (0, 1, 0, 1, 1, 0, 1, 1)  # per proj-tile: 1 -> Pool copy

_TC = tile.TileContext


def _legalize_waits(nc):
    """Split multi-wait sync_infos for this walrus build (1 wait/instr)."""
    uid = 0
    for f in nc.m.functions:
        for blk in f.blocks:
            insts = list(blk.instructions)
            out, changed = [], False
            for inst in insts:
                si = inst.sync_info
                cap = 2 if isinstance(inst, mybir.InstEventSemaphore) else 1
                if si is not None and len(si.on_wait) > cap:
                    changed = True
                    waits = list(si.on_wait)
                    for w in waits[:-cap]:
                        carrier = mybir.InstNoOp(
                            name=f"legwait-{uid}", engine=inst.engine,
                            ins=[], outs=[])
                        uid += 1
                        carrier.sync_info = bass_rust.SyncInfo(
                            on_wait=[w], on_update=[])
                        nc.register_instruction(carrier, overwrite=True)
                        out.append(carrier)
                    si.on_wait = waits[-cap:]
                    inst.sync_info = si
                out.append(inst)
            if changed:
                blk.instructions = out


def _bcast(ap, count):
    """Append a step-0 (broadcast) innermost free dim of `count`."""
    return bass.AP(tensor=ap.tensor, offset=ap.offset,
                   ap=[list(d) for d in ap.ap] + [[0, count]])


def _bcast_mid(ap, count, pos):
    """Insert a step-0 broadcast dim of `count` before ap dim `pos`."""
    dims = [list(d) for d in ap.ap]
    return bass.AP(tensor=ap.tensor, offset=ap.offset,
                   ap=dims[:pos] + [[0, count]] + dims[pos:])


def build_nc():
    nc = bass.Bass("TRN2", target_bir_lowering=False, debug=False)
    q_in = nc.dram_tensor("q", [B, 128, NT, D2], f16, kind="ExternalInput")
    k_in = nc.dram_tensor("k", [B, 128, NT, D2], f16, kind="ExternalInput")
    v_in = nc.dram_tensor("v", [B, 128, NT, HL, D + 1], f16,
                          kind="ExternalInput")
    cos_in = nc.dram_tensor("cos", [128, NT, HD], f16, kind="ExternalInput")
    sin_in = nc.dram_tensor("sin", [128, NT, HD], f16, kind="ExternalInput")
    wt_in = nc.dram_tensor("wt", [128, E], f16, kind="ExternalInput")
    out = nc.dram_tensor("out", [B * S, E], f16, kind="ExternalOutput")

    with _TC(nc) as tc:
        with tc.tile_pool(name="const", bufs=1) as cp, \
             tc.tile_pool(name="pa", bufs=2) as pa, \
             tc.tile_pool(name="pp", bufs=52) as pp, \
             tc.tile_pool(name="pb", bufs=2) as pb, \
             tc.tile_pool(name="ps_s", bufs=2, space="PSUM") as ps_s, \
             tc.tile_pool(name="ps_c", bufs=2, space="PSUM") as ps_c, \
             tc.tile_pool(name="ps_o", bufs=2, space="PSUM") as ps_o:
            ident = cp.tile([128, 128], f16, name="ident")
            make_identity(nc, ident)
            trimask = cp.tile([128, 128], f16, name="trimask")
            nc.vector.memset(trimask, 1.0)
            # keep where q - k >= 0 (upper triangle incl diag), else 0
            nc.gpsimd.affine_select(
                out=trimask, in_=trimask, compare_op=ALU.is_ge,
                fill=0.0, base=0, pattern=[[1, 128]], channel_multiplier=-1)
            epsb = cp.tile([128, 1], f32, name="epsb")
            nc.vector.memset(epsb, D * EPS)
            cos_sb = cp.tile([128, NT, HD], f16, name="cos_sb")
            sin_sb = cp.tile([128, NT, HD], f16, name="sin_sb")
            wt_sb = cp.tile([128, E], f16, name="wt_sb")
            qT = cp.tile([128, B, S], f16, name="qT")
            kT = cp.tile([128, B, S], f16, name="kT")
            vsb = cp.tile([128, B, NT, HL, D + 1], f16, name="vsb")
            rsk = cp.tile([128, B, NT, HL], f32, name="rsk")

            # ---------------- phase A: norm + rope + transpose ------------
            def phase_a(src, dstT, b, is_q, nsub=1):
                NS = NT // nsub
                for sub in range(nsub):
                    tsl = slice(sub * NS, (sub + 1) * NS)
                    raw = pa.tile([128, NS, HL, D], f16, tag="raw",
                                  name="raw")
                    nc.sync.dma_start(out=raw, in_=src.ap()[b][:, tsl])
                    sq = pa.tile([128, NS, HL, D], f16, tag="sq", name="sq")
                    nc.vector.tensor_mul(sq, raw, raw)
                    ss = pa.tile([128, NS, HL], f32, tag="ss", name="ss")
                    red = nc.gpsimd if REDUCE_ON_POOL else nc.vector
                    red.reduce_sum(ss, sq, axis=mybir.AxisListType.X)
                    # s64 = sqrt(ss + D*eps) = sqrt(D) * rms
                    s64 = pa.tile([128, NS, HL], f32, tag="s64", name="s64")
                    nc.scalar.activation(out=s64, in_=ss, func=ACTF.Sqrt,
                                         bias=epsb, scale=1.0)
                    if is_q:
                        rsq = pa.tile([128, NS, HL], f32, tag="rsq",
                                      name="rsq")
                        nc.vector.reciprocal(out=rsq, in_=s64)
                        rs8 = pa.tile([128, NS, HL], f16, tag="rs8",
                                      name="rs8")
                        # 1/rms = sqrt(D)/s64 -> x8
                        nc.vector.tensor_scalar_mul(rs8, rsq, 8.0)
                    else:
                        # exp scale = 1/(sqrt(D)*rms) = softmax scale / rms
                        nc.vector.reciprocal(out=rsk[:, b, tsl, :], in_=s64)
                    # rope on raw (rotation-invariant wrt the norm)
                    x1 = raw[:, :, :, 0:HD]
                    x2 = raw[:, :, :, HD:D]
                    cb = _bcast_mid(cos_sb[:, tsl], HL, 2)
                    sb = _bcast_mid(sin_sb[:, tsl], HL, 2)
                    t1 = pa.tile([128, NS, HL, HD], f16, tag="t1", name="t1")
                    t2 = pa.tile([128, NS, HL, HD], f16, tag="t2", name="t2")
                    rn = pa.tile([128, NS, HL, D], f16, tag="rn", name="rn")
                    nc.vector.tensor_mul(t1, x1, cb)
                    nc.vector.tensor_mul(t2, x2, sb)
                    nc.vector.tensor_sub(rn[:, :, :, 0:HD], t1, t2)
                    nc.vector.tensor_mul(t1, x1, sb)
                    nc.vector.tensor_mul(t2, x2, cb)
                    nc.vector.tensor_add(rn[:, :, :, HD:D], t1, t2)
                    if is_q:
                        nc.vector.tensor_mul(rn, rn, _bcast(rs8, D))
                    tsrc = rn
                    for g in range(NS // 8):
                        quad = ps_o.tile([128, 1024], f16, tag="o",
                                         name="quadA")
                        for tt in range(8):
                            t = g * 8 + tt
                            nc.tensor.transpose(
                                quad[:, tt * 128:(tt + 1) * 128],
                                tsrc[:, t], ident)
                        s0 = (sub * NS + g * 8) * 128
                        nc.vector.tensor_copy(
                            dstT[:, b, s0:s0 + 1024], quad)

            def load_v(b):
                nc.sync.dma_start(out=vsb[:, b], in_=v_in.ap()[b])

            # ---------------- phase B ------------------------------------
            p_tiles = {}

            def scores(b, ch):
                for hl in range(HL):
                    hsl = slice(hl * D, (hl + 1) * D)
                    for jj in range((ch + 1) * 8):
                        lo = max(jj * 128, ch * CW)
                        w = (ch + 1) * CW - lo
                        sps = ps_s.tile([128, CW], f32, tag="s", name="sps")
                        off = 0
                        while off < w:
                            pw = min(512, w - off)
                            nc.tensor.matmul(
                                sps[:, off:off + pw],
                                lhsT=kT[hsl, b, jj * 128:(jj + 1) * 128],
                                rhs=qT[hsl, b, lo + off:lo + off + pw],
                                start=True, stop=True)
                            off += pw
                        pt = pp.tile([128, CW], f16, tag="p", name="pt")
                        nc.scalar.activation(
                            out=pt[:, 0:w], in_=sps[:, 0:w], func=ACTF.Exp,
                            scale=rsk[:, b, jj, hl:hl + 1])
                        if lo == jj * 128:
                            dg = nc.gpsimd if DIAG_ON_POOL else nc.vector
                            dg.tensor_mul(pt[:, 0:128], pt[:, 0:128],
                                          trimask)
                        p_tiles[(b, ch, hl, jj)] = pt

            def ctx_proj(b, ch, halves=(0, 1), emit_proj=True):
                cpairs = []
                for half in halves:
                    cpr = pb.tile([128, 4, HL, D], f16, tag="cp", bufs=4,
                                  name="cpr")
                    for hl in range(HL):
                        ctx = ps_c.tile([128, 4, D + 1], f32, tag="c",
                                        name="ctx")
                        for ql in range(4):
                            ig = ch * 8 + half * 4 + ql
                            for jj in range(ig + 1):
                                pt = p_tiles[(b, ch, hl, jj)]
                                lo = max(jj * 128, ch * CW)
                                col = ig * 128 - lo
                                nc.tensor.matmul(
                                    ctx[:, ql, :],
                                    lhsT=pt[:, col:col + 128],
                                    rhs=vsb[:, b, jj, hl, :],
                                    start=(jj == 0), stop=(jj == ig))
                        rsr = pb.tile([128, 4], f32, tag="rsr", bufs=4,
                                      name="rsr")
                        nc.vector.reciprocal(out=rsr, in_=ctx[:, :, D])
                        nc.vector.tensor_mul(
                            cpr[:, :, hl, :], ctx[:, :, 0:D],
                            _bcast(rsr, D))
                    cpairs.append((half, cpr))
                if emit_proj:
                    proj(b, ch, cpairs)
                return cpairs

            def proj(b, ch, cpairs):
                for half, cpr in cpairs:
                    quad = ps_o.tile([128, 512], f16, tag="o", name="quadT")
                    for ql in range(4):
                        nc.tensor.transpose(
                            quad[:, ql * 128:(ql + 1) * 128],
                            cpr[:, ql], ident)
                    ctxT = pb.tile([128, 4, 128], f16, tag="ctxT",
                                   name="ctxT")
                    nc.vector.tensor_copy(ctxT, quad)
                    for ql in range(4):
                        ig = ch * 8 + half * 4 + ql
                        osb = pb.tile([128, E], f16, tag="osb", bufs=3,
                                      name="osb")
                        for nn in range(2):
                            po = ps_o.tile([128, 512], f32, tag="o",
                                           name="po")
                            nc.tensor.matmul(
                                po, lhsT=ctxT[:, ql],
                                rhs=wt_sb[:, nn * 512:(nn + 1) * 512],
                                start=True, stop=True)
                            eng = (nc.gpsimd
                                   if OSB_POOL_PATTERN[(ig * 2 + nn)
                                                       % len(OSB_POOL_PATTERN)]
                                   else nc.vector)
                            eng.tensor_copy(osb[:, nn * 512:(nn + 1) * 512],
                                            po)
                        row0 = b * S + ig * 128
                        nc.sync.dma_start(out=out.ap()[row0:row0 + 128, :],
                                          in_=osb)

            # ---------------- schedule -----------------------------------
            nc.sync.dma_start(out=cos_sb, in_=cos_in.ap())
            nc.sync.dma_start(out=sin_sb, in_=sin_in.ap())
            phase_a(k_in, kT, 0, False, nsub=2)
            phase_a(q_in, qT, 0, True, nsub=2)
            nc.sync.dma_start(out=wt_sb, in_=wt_in.ap())
            load_v(0)
            scores(0, 0)
            scores(0, 1)
            ctx_proj(0, 0)
            phase_a(k_in, kT, 1, False)
            phase_a(q_in, qT, 1, True)
            load_v(1)
            scores(1, 0)
            ctx_proj(0, 1)
            scores(1, 1)
            ctx_proj(1, 0)
            ctx_proj(1, 1)
    _legalize_waits(nc)
    return nc


# even rope lanes first, then odd (consistent perm leaves q.k unchanged)
_ROPE_PERM = np.concatenate([np.arange(0, D, 2), np.arange(1, D, 2)])


def _shard_inputs(q, k, v, cos, sin, proj_w):
    """Per-core input maps (host-side layout/dtype prep only)."""
    qh = q.reshape(B, S, H, D)
    kh = k.reshape(B, S, H, D)
    vh = v.reshape(B, S, H, D)
    cos_t = np.ascontiguousarray(
        cos.reshape(NT, 128, HD).transpose(1, 0, 2), np.float16)
    sin_t = np.ascontiguousarray(
        sin.reshape(NT, 128, HD).transpose(1, 0, 2), np.float16)
    maps = []
    for core in range(N_CORES):
        hs = slice(HL * core, HL * (core + 1))

        def pack_qk(x):
            xs = x[:, :, hs, :][..., _ROPE_PERM]      # [B, S, HL, D]
            xs = xs.reshape(B, NT, 128, HL * D)
            return np.ascontiguousarray(
                xs.transpose(0, 2, 1, 3), np.float16)  # [B, 128, NT, D2]

        vs = vh[:, :, hs, :]                           # [B, S, HL, D]
        vcat = np.concatenate(
            [vs, np.ones((B, S, HL, 1), vs.dtype)], axis=-1)
        v_map = np.ascontiguousarray(
            vcat.reshape(B, NT, 128, HL, D + 1).transpose(0, 2, 1, 3, 4),
            np.float16)                                # [B, 128, NT, HL, 65]
        wt_c = np.ascontiguousarray(
            proj_w[:, 128 * core:128 * (core + 1)].T, np.float16)
        maps.append({
            "q": pack_qk(qh), "k": pack_qk(kh), "v": v_map,
            "cos": cos_t, "sin": sin_t, "wt": wt_c,
        })
    return maps


_NC_CACHE = []


def _get_nc():
    if not _NC_CACHE:
        _NC_CACHE.append(build_nc())
    return _NC_CACHE[0]


def kernel(q, k, v, attn_mask, padding_mask, qn_w, kn_w, proj_w, proj_b,
           cos, sin):
    q = np.asarray(q, np.float32)
    k = np.asarray(k, np.float32)
    v = np.asarray(v, np.float32)
    proj_w = np.asarray(proj_w, np.float32)
    proj_b = np.asarray(proj_b, np.float32)
    cos = np.asarray(cos, np.float32)
    sin = np.asarray(sin, np.float32)
    attn_mask = np.asarray(attn_mask)
    padding_mask = np.asarray(padding_mask)
    qn_w = np.asarray(qn_w, np.float32)
    kn_w = np.asarray(kn_w, np.float32)
    # The kernel bakes in: causal attn_mask, no padding, unit RMSNorm weights.
    assert np.array_equal(
        attn_mask.reshape(S, S), np.tril(np.ones((S, S), attn_mask.dtype)))
    assert padding_mask.all()
    assert np.all(qn_w == 1.0) and np.all(kn_w == 1.0)

    in_maps = _shard_inputs(q, k, v, cos, sin, proj_w)
    nc = _get_nc()
    res = run_bass_kernel_spmd(nc, in_maps, core_ids=list(range(N_CORES)))
    parts = np.stack([r["out"] for r in res.results])      # [8, B*S, E] f16
    full = parts.astype(np.float32).sum(axis=0) + proj_b[None, :]
    return full.reshape(B, S, E).astype(np.float32)
